# revision 22
# baseline (speedup 1.0000x reference)
"""Multi-head attention with per-pair relative embeddings (nn_MultiHeadAttention),
SPMD across 8 TRN2 NeuronCores.

Reference computation (B=2, N=M=512, C=256, H=8, HC=32):
    q = (input_q @ Wq.T + bq)   -> (B,H,N,HC)
    k = (input_k @ Wk.T + bk)   -> (B,H,M,HC)
    v = (input_v @ Wv.T + bv)   -> (B,H,M,HC)
    p = (embed_qk @ Wp.T + bp)  -> (B,N,M,H,HC)
    scores = (einsum('bhnc,bnmhc->bhnm', q, p) + einsum('bhnc,bhmc->bhnm', q, k)) / sqrt(HC)
    scores = scores * attention_factors[:,None] * key_weights[:,None,None,:]
    scores = where(key_masks, -inf, scores)
    attn = softmax(scores, -1)
    out = einsum('bhnm,bhmc->bhnc', attn, v) -> (B, N, C)

Key algebraic rewrite (32x FLOP reduction): never materialize p.  With
qhat = q / sqrt(HC):
    scores_p[h,n,m] = sum_d qp[h,n,d] * embed_qk[n,m,d],
        qp[h,n,d] = sum_hc qhat[h*32+hc, n] * Wp[h*32+hc, d]
    scores_e[h,n,m] = sum_c qk[h,n,c] * input_k[m,c],
        qk[h,n,c] = sum_hc qhat[h*32+hc, n] * Wk[h*32+hc, c]
plus a per-(h,n) constant  qhat . (bp+bk)_headslice  which is handled as an
additive bias (zero for the given inputs, still implemented generically).

Sharding: core = b*4 + (block of 128 query rows n).  Each core streams its
67MB embed_qk slice (host-pretransposed to (n, C, M) so the contraction dim C
lands on SBUF partitions) and computes scores / softmax / attn@v locally.
"""

import math
import os
import sys

import numpy as np

sys.path.insert(0, "/opt/trn_rl_repo")

B, N, M, C, H = 2, 512, 512, 256, 8
HC = C // H
NCORES = 8
NL = N * B // NCORES  # 128 query rows per core
NG = 16               # n rows per softmax group (16*8 heads = 128 partitions)
GROUPS = NL // NG     # 8 groups
SCALE = 1.0 / math.sqrt(HC)

# set by test.py for profiling runs
TRACE = False
LAST_RESULT = None

_PROGRAM_CACHE = {}


def _build_program(bias_on: bool, pen_on: bool):
    """Build the (SPMD, identical-per-core) Bass program."""
    import concourse.bacc as bacc
    import concourse.bass as bass
    import concourse.tile as tile
    from concourse import mybir
    from concourse.masks import make_identity

    f32 = mybir.dt.float32
    bf16 = mybir.dt.bfloat16
    AF = mybir.ActivationFunctionType

    nc = bacc.Bacc(
        "TRN2",
        target_bir_lowering=False,
        debug=False,
        enable_asserts=True,
        num_devices=NCORES,
    )

    def din(name, shape, dt=f32):
        return nc.dram_tensor(name, list(shape), dt, kind="ExternalInput").ap()

    embedT_d = din("embedT", (C, NL, M), bf16)     # per-core embed slice, (d, n, m)
    xqT_d = din("xqT", (C, NL), bf16)              # input_q slice, transposed
    xkT_d = din("xkT", (C, M), bf16)               # input_k (this core's b), transposed
    xvT_d = din("xvT", (C, M), bf16)               # input_v (this core's b), transposed
    af_d = din("af", (NL, M))                # attention_factors * key_weights
    wqT_d = din("WqT", (C, C), bf16)               # Wq.T  (cin, cout)
    wkr_d = din("Wkr", (HC, H, C), bf16)           # Wk[h*HC+hc, c] as [hc, h, c]
    wpr_d = din("Wpr", (HC, H, C), bf16)           # Wp[h*HC+hc, d] as [hc, h, d]
    wvT_d = din("WvT", (C, C), bf16)               # Wv.T  (cin, cout)
    bqs_d = din("bqs", (HC, H))              # (bq * SCALE)[h*HC+hc] as [hc, h]
    bv_d = din("bvb", (128, C))              # bv broadcast to 128 partitions
    if bias_on:
        bpb_d = din("bpb", (HC, H))          # (bp+bk)[h*HC+hc] as [hc, h]
    if pen_on:
        pen_d = din("pen", (1, M))           # -1e30 where key_mask else 0
    out_d = nc.dram_tensor("out", [NL, C], f32, kind="ExternalOutput").ap()

    def r(ap):
        return ap

    with tile.TileContext(nc) as tc:
        with (
            tc.tile_pool(name="consts", bufs=1) as consts,
            tc.tile_pool(name="embed", bufs=6) as embed_pool,
            tc.tile_pool(name="work", bufs=2) as work,
            tc.tile_pool(name="small", bufs=2) as small,
        ):
            # ---- load constants ----
            ident = consts.tile([128, 128], bf16)
            make_identity(nc, ident)

            def load2(dram, cols, tag, dt=f32):
                ts = []
                for ch in range(2):
                    t = consts.tile([128, cols], dt, tag=f"{tag}{ch}", name=f"{tag}{ch}")
                    nc.sync.dma_start(out=t, in_=dram[128 * ch:128 * ch + 128, :])
                    ts.append(t)
                return ts

            wqT_sb = load2(wqT_d, C, "wqT", bf16)
            wvT_sb = load2(wvT_d, C, "wvT", bf16)
            xqT_sb = load2(xqT_d, NL, "xqT", bf16)
            xkT_sb = load2(xkT_d, M, "xkT", bf16)
            xvT_sb = load2(xvT_d, M, "xvT", bf16)
            wkr_sb = consts.tile([HC, H, C], bf16)
            nc.sync.dma_start(out=wkr_sb, in_=wkr_d)
            wpr_sb = consts.tile([HC, H, C], bf16)
            nc.sync.dma_start(out=wpr_sb, in_=wpr_d)
            bqs_sb = consts.tile([HC, H], f32)
            nc.sync.dma_start(out=bqs_sb, in_=bqs_d)
            bv_sb = consts.tile([128, C], f32)
            nc.sync.dma_start(out=bv_sb, in_=bv_d)
            if bias_on:
                bpb_sb = consts.tile([HC, H], f32)
                nc.sync.dma_start(out=bpb_sb, in_=bpb_d)
            if pen_on:
                pen_sb = consts.tile([128, M], f32)
                nc.sync.dma_start(
                    out=pen_sb,
                    in_=bass.AP(tensor=pen_d.tensor, offset=pen_d.offset,
                                ap=[[0, 128], [1, M]]),
                )

            qhat_sb = consts.tile([HC, H, NL], bf16)
            qp_dense = [consts.tile([128, H, NL], bf16, tag=f"qd{i}", name=f"qd{i}") for i in range(2)]
            # qp_pad[*, s, h, n] = qp[h, n] if s == n%4 else 0 — a zero-padded
            # i=32 stationary operand so each n's scores_p matmul can target a
            # 32-aligned PSUM quadrant while touching only its own 8 rows.
            qp_pad = [consts.tile([128, NL, 4, H], bf16, tag=f"qp{i}", name=f"qp{i}") for i in range(2)]
            qk_sb = [consts.tile([128, NL, H], bf16, tag=f"qk{i}", name=f"qk{i}") for i in range(2)]
            v_sb = [consts.tile([128, C], bf16, tag=f"v{i}", name=f"v{i}") for i in range(4)]
            attnT_sb = [consts.tile([128, NL, H], bf16, tag=f"at{i}", name=f"at{i}") for i in range(4)]
            hid_sb = consts.tile([128, C], f32)
            if bias_on:
                bias_nh = consts.tile([NL, H], f32)

            # ---- q projection: qhat[hc, h, n] = ((xq @ Wq.T + bq) * s).T ----
            with tc.tile_pool(name="ps_q", bufs=2, space="PSUM") as psq:
                for h in range(H):
                    qt = psq.tile([HC, NL], f32, tag="qt")
                    for ci in range(2):
                        nc.tensor.matmul(
                            qt,
                            lhsT=r(wqT_sb[ci][:, HC * h:HC * h + HC]),
                            rhs=r(xqT_sb[ci]),
                            start=(ci == 0), stop=(ci == 1),
                        )
                    nc.scalar.activation(
                        out=qhat_sb[:, h, :], in_=qt, func=AF.Identity,
                        bias=bqs_sb[:, h:h + 1], scale=SCALE,
                    )

            # ---- qp / qk: per-head contraction of qhat with Wp / Wk rows ----
            nc.vector.memset(qp_pad[0], 0.0)
            nc.vector.memset(qp_pad[1], 0.0)
            with tc.tile_pool(name="ps_qpk", bufs=2, space="PSUM") as pqpk:
                for h in range(H):
                    rq = qhat_sb[:, h, :]  # (HC, NL)
                    for dc in range(2):
                        tk = pqpk.tile([128, NL], f32, tag="tk")
                        nc.tensor.matmul(
                            tk,
                            lhsT=r(wkr_sb[:, h, 128 * dc:128 * dc + 128]),
                            rhs=r(rq), start=True, stop=True,
                        )
                        nc.scalar.copy(out=qk_sb[dc][:, :, h], in_=tk)
                        tp = pqpk.tile([128, NL], f32, tag="tp")
                        nc.tensor.matmul(
                            tp,
                            lhsT=r(wpr_sb[:, h, 128 * dc:128 * dc + 128]),
                            rhs=r(rq), start=True, stop=True,
                        )
                        nc.scalar.copy(out=qp_dense[dc][:, h, :], in_=tp)
                # scatter qp_dense -> qp_pad diagonal slots via 4 DMAs per chunk
                for dc in range(2):
                    qd = qp_dense[dc]
                    qpp = qp_pad[dc]
                    for s in range(4):
                        for h in range(H):
                            dst = bass.AP(
                                tensor=qpp.tensor,
                                offset=qpp.offset + 40 * s + h,
                                ap=[qpp.ap[0], [128, 32]])
                            srcap = bass.AP(
                                tensor=qd.tensor,
                                offset=qd.offset + h * NL + s,
                                ap=[qd.ap[0], [4, 32]])
                            nc.gpsimd.dma_start(out=dst, in_=srcap)

            # ---- v projection: v[m, cout] (+ bv) ----
            with tc.tile_pool(name="ps_v", bufs=2, space="PSUM") as psv:
                for mc in range(4):
                    vt = psv.tile([128, C], f32, tag="vt")
                    for ci in range(2):
                        nc.tensor.matmul(
                            vt,
                            lhsT=r(xvT_sb[ci][:, 128 * mc:128 * mc + 128]),
                            rhs=r(wvT_sb[ci]),
                            start=(ci == 0), stop=(ci == 1),
                        )
                    nc.vector.tensor_add(v_sb[mc], vt, bv_sb)

            # ---- per-(h,n) additive bias: qhat . (bp+bk)_headslice ----
            if bias_on:
                with tc.tile_pool(name="ps_b", bufs=1, space="PSUM") as psb:
                    bt = psb.tile([NL, H], f32)
                    for h in range(H):
                        nc.tensor.matmul(
                            bt[:, h:h + 1], lhsT=r(qhat_sb[:, h, :]),
                            rhs=r(bpb_sb[:, h:h + 1]),
                            start=True, stop=True,
                        )
                    nc.scalar.copy(out=bias_nh, in_=bt)

            # ---- main loop: scores -> softmax -> transposed attn ----
            with (
                tc.tile_pool(name="ps_sc", bufs=2, space="PSUM") as ps_sc,
                tc.tile_pool(name="ps_tr", bufs=4, space="PSUM") as ps_tr,
                tc.tile_pool(name="ps_hid", bufs=1, space="PSUM") as ps_hid,
            ):
                hid = ps_hid.tile([128, C], f32)
                for g in range(GROUPS):
                    s_ps = ps_sc.tile([128, M], f32, tag="scores")
                    for q in range(NG // 4):
                        # quadrant q: 4 consecutive n rows -> PSUM rows 32q..32q+32
                        n0 = g * NG + q * 4
                        if q % 2 == 0:
                            # one DMA pair fetches 8 n rows (2 quadrants); the
                            # d-major DRAM layout gives 8KB contiguous rows
                            et = []
                            for ch in range(2):
                                t = embed_pool.tile([128, 8, M], bf16, tag=f"e{ch}", name=f"e{ch}")
                                src = bass.AP(
                                    tensor=embedT_d.tensor,
                                    offset=embedT_d.offset + ch * 128 * NL * M + n0 * M,
                                    ap=[[NL * M, 128], [M, 8], [1, M]],
                                )
                                nc.sync.dma_start(out=t, in_=src)
                                et.append(t)
                        o = s_ps[32 * q:32 * q + 32, :]
                        tpos = (0, 32 * q)
                        # scores_e: all 4 n's share the xkT rhs -> dense i=32
                        for ch in range(2):
                            nc.tensor.matmul(o, lhsT=r(qk_sb[ch][:, n0:n0 + 4, :]),
                                             rhs=r(xkT_sb[ch]),
                                             start=(ch == 0), stop=False,
                                             tile_position=tpos)
                        # scores_p: per-n, zero-padded i=32 accumulates onto
                        # that n's 8 rows only
                        for i in range(4):
                            n = n0 + i
                            for ch in range(2):
                                nc.tensor.matmul(
                                    o, lhsT=r(qp_pad[ch][:, n, :, :]),
                                    rhs=r(et[ch][:, (q % 2) * 4 + i, :]),
                                    start=False, stop=(i == 3 and ch == 1),
                                    tile_position=tpos)

                    # softmax over m (free dim), 16 n x 8 h on partitions
                    af_t = work.tile([128, M], f32, tag="af")
                    nc.gpsimd.dma_start(
                        out=af_t,
                        in_=bass.AP(tensor=af_d.tensor,
                                    offset=af_d.offset + g * NG * M,
                                    ap=[[M, NG], [0, H], [1, M]]),
                    )
                    tt = work.tile([128, M], f32, tag="T")
                    if bias_on:
                        bias_grp = small.tile([128, 1], f32, tag="bgrp")
                        nc.gpsimd.dma_start(out=bias_grp,
                                          in_=bias_nh[g * NG:(g + 1) * NG, :])
                        nc.vector.tensor_scalar(out=tt, in0=s_ps, scalar1=bias_grp,
                                                scalar2=None,
                                                op0=mybir.AluOpType.add)
                        nc.vector.tensor_mul(tt, tt, af_t)
                    else:
                        nc.vector.tensor_mul(tt, s_ps, af_t)
                    if pen_on:
                        nc.vector.tensor_add(tt, tt, pen_sb)
                    negmax = small.tile([128, 1], f32, tag="negmax")
                    nc.vector.reduce_max(negmax, tt, axis=mybir.AxisListType.X,
                                         negate=True)
                    ex = work.tile([128, M], bf16, tag="ex")
                    sums = small.tile([128, 1], f32, tag="sums")
                    nc.scalar.activation(out=ex, in_=tt, func=AF.Exp,
                                         bias=negmax, scale=1.0, accum_out=sums)
                    rcp = small.tile([128, 1], f32, tag="rcp")
                    nc.vector.reciprocal(rcp, sums)
                    nc.vector.tensor_scalar(out=ex, in0=ex, scalar1=rcp,
                                            scalar2=None,
                                            op0=mybir.AluOpType.mult)
                    for mc in range(4):
                        tp = ps_tr.tile([128, 128], bf16, tag="tp")
                        nc.tensor.transpose(tp, ex[:, 128 * mc:128 * mc + 128],
                                            ident)
                        nc.scalar.copy(
                            out=attnT_sb[mc][:, g * NG:(g + 1) * NG, :], in_=tp)
                    if g % 2 == 1:
                        # attn @ v for this pair of groups (32 query rows)
                        pr = g // 2
                        for h in range(H):
                            for mc in range(4):
                                nc.tensor.matmul(
                                    hid[32 * pr:32 * pr + 32, 32 * h:32 * h + 32],
                                    lhsT=r(attnT_sb[mc][:, 32 * pr:32 * pr + 32, h]),
                                    rhs=r(v_sb[mc][:, 32 * h:32 * h + 32]),
                                    start=(mc == 0), stop=(mc == 3),
                                    tile_position=(0, 32 * pr),
                                )

                nc.scalar.copy(out=hid_sb, in_=hid)
                nc.sync.dma_start(out=out_d, in_=hid_sb)

    nc.compile()
    return nc


def _get_program(bias_on: bool, pen_on: bool):
    key = (bias_on, pen_on)
    if key not in _PROGRAM_CACHE:
        _PROGRAM_CACHE[key] = _build_program(bias_on, pen_on)
    return _PROGRAM_CACHE[key]


def kernel(input_q, input_k, input_v, embed_qk, key_weights, key_masks,
           attention_factors, Wq, bq, Wk, bk, Wv, bv, Wp, bp):
    global LAST_RESULT
    from concourse.bass_utils import run_bass_kernel_spmd

    f = np.float32
    input_q = np.asarray(input_q, f)
    input_k = np.asarray(input_k, f)
    input_v = np.asarray(input_v, f)
    embed_qk = np.asarray(embed_qk, f)
    key_weights = np.asarray(key_weights, f)
    key_masks = np.asarray(key_masks)
    attention_factors = np.asarray(attention_factors, f)
    Wq, bq, Wk, bk, Wv, bv, Wp, bp = (np.asarray(x, f) for x in
                                      (Wq, bq, Wk, bk, Wv, bv, Wp, bp))

    bias_on = bool(np.any(bp + bk != 0.0))
    pen_on = bool(np.any(key_masks))
    nc = _get_program(bias_on, pen_on)

    # host-side staging (layout + bf16 narrowing for the matmul path)
    import ml_dtypes
    bf = ml_dtypes.bfloat16
    wqT = np.ascontiguousarray(Wq.T).astype(bf)
    wvT = np.ascontiguousarray(Wv.T).astype(bf)
    wkr = np.ascontiguousarray(Wk.reshape(H, HC, C).transpose(1, 0, 2)).astype(bf)
    wpr = np.ascontiguousarray(Wp.reshape(H, HC, C).transpose(1, 0, 2)).astype(bf)
    bqs = np.ascontiguousarray((bq * SCALE).reshape(H, HC).T)
    bvb = np.ascontiguousarray(np.broadcast_to(bv[None, :], (128, C)))
    if bias_on:
        bpb = np.ascontiguousarray((bp + bk).reshape(H, HC).T)
    if pen_on:
        pen = np.where(key_masks, np.float32(-1e30), np.float32(0.0)).astype(f)

    af_eff = attention_factors * key_weights[:, None, :]  # (B, N, M)

    in_maps = []
    for core in range(NCORES):
        b = core // (NCORES // B)
        n0 = (core % (NCORES // B)) * NL
        m = {
            "embedT": np.ascontiguousarray(
                embed_qk[b, n0:n0 + NL].transpose(2, 0, 1)).astype(bf),
            "xqT": np.ascontiguousarray(input_q[b, n0:n0 + NL].T).astype(bf),
            "xkT": np.ascontiguousarray(input_k[b].T).astype(bf),
            "xvT": np.ascontiguousarray(input_v[b].T).astype(bf),
            "af": np.ascontiguousarray(af_eff[b, n0:n0 + NL]),
            "WqT": wqT, "Wkr": wkr, "Wpr": wpr, "WvT": wvT,
            "bqs": bqs, "bvb": bvb,
        }
        if bias_on:
            m["bpb"] = bpb
        if pen_on:
            m["pen"] = np.ascontiguousarray(pen[b].reshape(1, M))
        in_maps.append(m)

    res = run_bass_kernel_spmd(nc, in_maps, core_ids=list(range(NCORES)),
                               trace=TRACE)
    LAST_RESULT = res

    out = np.empty((B, N, C), f)
    for core in range(NCORES):
        b = core // (NCORES // B)
        n0 = (core % (NCORES // B)) * NL
        out[b, n0:n0 + NL] = res.results[core]["out"]
    return out


# revision 23
# speedup vs baseline: 2.7451x; 2.7451x over previous
"""Multi-head attention with per-pair relative embeddings (nn_MultiHeadAttention),
SPMD across 8 TRN2 NeuronCores.

Reference computation (B=2, N=M=512, C=256, H=8, HC=32):
    q = (input_q @ Wq.T + bq)   -> (B,H,N,HC)
    k = (input_k @ Wk.T + bk)   -> (B,H,M,HC)
    v = (input_v @ Wv.T + bv)   -> (B,H,M,HC)
    p = (embed_qk @ Wp.T + bp)  -> (B,N,M,H,HC)
    scores = (einsum('bhnc,bnmhc->bhnm', q, p) + einsum('bhnc,bhmc->bhnm', q, k)) / sqrt(HC)
    scores = scores * attention_factors[:,None] * key_weights[:,None,None,:]
    scores = where(key_masks, -inf, scores)
    attn = softmax(scores, -1)
    out = einsum('bhnm,bhmc->bhnc', attn, v) -> (B, N, C)

Key algebraic rewrite (32x FLOP reduction): never materialize p.  With
qhat = q / sqrt(HC):
    scores_p[h,n,m] = sum_d qp[h,n,d] * embed_qk[n,m,d],
        qp[h,n,d] = sum_hc qhat[h*32+hc, n] * Wp[h*32+hc, d]
    scores_e[h,n,m] = sum_c qk[h,n,c] * input_k[m,c],
        qk[h,n,c] = sum_hc qhat[h*32+hc, n] * Wk[h*32+hc, c]
plus a per-(h,n) constant  qhat . (bp+bk)_headslice  which is handled as an
additive bias (zero for the given inputs, still implemented generically).

Sharding: core = b*4 + (block of 128 query rows n).  Each core streams its
67MB embed_qk slice (host-pretransposed to (n, C, M) so the contraction dim C
lands on SBUF partitions) and computes scores / softmax / attn@v locally.
"""

import math
import os
import sys

import numpy as np

sys.path.insert(0, "/opt/trn_rl_repo")

B, N, M, C, H = 2, 512, 512, 256, 8
HC = C // H
NCORES = 8
NL = N * B // NCORES  # 128 query rows per core
NG = 16               # n rows per softmax group (16*8 heads = 128 partitions)
GROUPS = NL // NG     # 8 groups
SCALE = 1.0 / math.sqrt(HC)

# set by test.py for profiling runs
TRACE = False
LAST_RESULT = None

_PROGRAM_CACHE = {}


def _build_program(bias_on: bool, pen_on: bool):
    """Build the (SPMD, identical-per-core) Bass program."""
    import concourse.bacc as bacc
    import concourse.bass as bass
    import concourse.tile as tile
    from concourse import mybir
    from concourse.masks import make_identity

    f32 = mybir.dt.float32
    bf16 = mybir.dt.bfloat16
    AF = mybir.ActivationFunctionType

    nc = bacc.Bacc(
        "TRN2",
        target_bir_lowering=False,
        debug=False,
        enable_asserts=True,
        num_devices=NCORES,
    )

    def din(name, shape, dt=f32):
        return nc.dram_tensor(name, list(shape), dt, kind="ExternalInput").ap()

    embedT_d = din("embedT", (C, NL, M), bf16)     # per-core embed slice, (d, n, m)
    xqT_d = din("xqT", (C, NL), bf16)              # input_q slice, transposed
    xkT_d = din("xkT", (C, M), bf16)               # input_k (this core's b), transposed
    xvT_d = din("xvT", (C, M), bf16)               # input_v (this core's b), transposed
    af_d = din("af", (NL, M))                # attention_factors * key_weights
    wqT_d = din("WqT", (C, C), bf16)               # Wq.T  (cin, cout)
    wkr_d = din("Wkr", (HC, H, C), bf16)           # Wk[h*HC+hc, c] as [hc, h, c]
    wpr_d = din("Wpr", (HC, H, C), bf16)           # Wp[h*HC+hc, d] as [hc, h, d]
    wvT_d = din("WvT", (C, C), bf16)               # Wv.T  (cin, cout)
    bqs_d = din("bqs", (HC, H))              # (bq * SCALE)[h*HC+hc] as [hc, h]
    bv_d = din("bvb", (128, C))              # bv broadcast to 128 partitions
    if bias_on:
        bpb_d = din("bpb", (HC, H))          # (bp+bk)[h*HC+hc] as [hc, h]
    if pen_on:
        pen_d = din("pen", (1, M))           # -1e30 where key_mask else 0
    out_d = nc.dram_tensor("out", [NL, C], f32, kind="ExternalOutput").ap()

    def r(ap):
        return ap

    with tile.TileContext(nc) as tc:
        with (
            tc.tile_pool(name="consts", bufs=1) as consts,
            tc.tile_pool(name="embed", bufs=6) as embed_pool,
            tc.tile_pool(name="work", bufs=2) as work,
            tc.tile_pool(name="small", bufs=2) as small,
        ):
            # ---- load constants ----
            ident = consts.tile([128, 128], bf16)
            make_identity(nc, ident)

            def load2(dram, cols, tag, dt=f32):
                ts = []
                for ch in range(2):
                    t = consts.tile([128, cols], dt, tag=f"{tag}{ch}", name=f"{tag}{ch}")
                    nc.sync.dma_start(out=t, in_=dram[128 * ch:128 * ch + 128, :])
                    ts.append(t)
                return ts

            wqT_sb = load2(wqT_d, C, "wqT", bf16)
            wvT_sb = load2(wvT_d, C, "wvT", bf16)
            xqT_sb = load2(xqT_d, NL, "xqT", bf16)
            xkT_sb = load2(xkT_d, M, "xkT", bf16)
            xvT_sb = load2(xvT_d, M, "xvT", bf16)
            wkr_sb = consts.tile([HC, H, C], bf16)
            nc.sync.dma_start(out=wkr_sb, in_=wkr_d)
            wpr_sb = consts.tile([HC, H, C], bf16)
            nc.sync.dma_start(out=wpr_sb, in_=wpr_d)
            bqs_sb = consts.tile([HC, H], f32)
            nc.sync.dma_start(out=bqs_sb, in_=bqs_d)
            bv_sb = consts.tile([128, C], f32)
            nc.sync.dma_start(out=bv_sb, in_=bv_d)
            if bias_on:
                bpb_sb = consts.tile([HC, H], f32)
                nc.sync.dma_start(out=bpb_sb, in_=bpb_d)
            if pen_on:
                pen_sb = consts.tile([128, M], f32)
                nc.sync.dma_start(
                    out=pen_sb,
                    in_=bass.AP(tensor=pen_d.tensor, offset=pen_d.offset,
                                ap=[[0, 128], [1, M]]),
                )

            qhat_sb = consts.tile([HC, H, NL], bf16)
            # qp_pad[*, s, h, n] = qp[h, n] if s == n%4 else 0 — a zero-padded
            # i=32 stationary operand so each n's scores_p matmul can target a
            # 32-aligned PSUM quadrant while touching only its own 8 rows.
            qp_pad = [consts.tile([128, NL, 4, H], bf16, tag=f"qp{i}", name=f"qp{i}") for i in range(2)]
            qk_sb = [consts.tile([128, NL, H], bf16, tag=f"qk{i}", name=f"qk{i}") for i in range(2)]
            v_sb = [consts.tile([128, C], bf16, tag=f"v{i}", name=f"v{i}") for i in range(4)]
            attnT_sb = [consts.tile([128, NL, H], bf16, tag=f"at{i}", name=f"at{i}") for i in range(4)]
            hid_sb = consts.tile([128, C], f32)
            if bias_on:
                bias_nh = consts.tile([NL, H], f32)

            # ---- q projection: qhat[hc, h, n] = ((xq @ Wq.T + bq) * s).T ----
            with tc.tile_pool(name="ps_q", bufs=2, space="PSUM") as psq:
                for h in range(H):
                    qt = psq.tile([HC, NL], f32, tag="qt")
                    for ci in range(2):
                        nc.tensor.matmul(
                            qt,
                            lhsT=r(wqT_sb[ci][:, HC * h:HC * h + HC]),
                            rhs=r(xqT_sb[ci]),
                            start=(ci == 0), stop=(ci == 1),
                        )
                    nc.scalar.activation(
                        out=qhat_sb[:, h, :], in_=qt, func=AF.Identity,
                        bias=bqs_sb[:, h:h + 1], scale=SCALE,
                    )

            # ---- qp / qk: per-head contraction of qhat with Wp / Wk rows ----
            nc.vector.memset(qp_pad[0], 0.0)
            nc.vector.memset(qp_pad[1], 0.0)
            with tc.tile_pool(name="ps_qpk", bufs=2, space="PSUM") as pqpk:
                for h in range(H):
                    rq = qhat_sb[:, h, :]  # (HC, NL)
                    for dc in range(2):
                        tk = pqpk.tile([128, NL], f32, tag="tk")
                        nc.tensor.matmul(
                            tk,
                            lhsT=r(wkr_sb[:, h, 128 * dc:128 * dc + 128]),
                            rhs=r(rq), start=True, stop=True,
                        )
                        nc.scalar.copy(out=qk_sb[dc][:, :, h], in_=tk)
                        tp = pqpk.tile([128, NL], f32, tag="tp")
                        nc.tensor.matmul(
                            tp,
                            lhsT=r(wpr_sb[:, h, 128 * dc:128 * dc + 128]),
                            rhs=r(rq), start=True, stop=True,
                        )
                        tp4 = tp.rearrange("p (n4 s) -> p n4 s", s=4)
                        dst4 = qp_pad[dc][:, :, :, h].rearrange(
                            "p (n4 s2) s -> p n4 s2 s", s2=4)
                        for s in range(4):
                            eng = nc.scalar if s < 2 else nc.vector
                            if s < 2:
                                nc.scalar.copy(out=dst4[:, :, s, s], in_=tp4[:, :, s])
                            else:
                                nc.vector.tensor_copy(out=dst4[:, :, s, s], in_=tp4[:, :, s])

            # ---- v projection: v[m, cout] (+ bv) ----
            with tc.tile_pool(name="ps_v", bufs=2, space="PSUM") as psv:
                for mc in range(4):
                    vt = psv.tile([128, C], f32, tag="vt")
                    for ci in range(2):
                        nc.tensor.matmul(
                            vt,
                            lhsT=r(xvT_sb[ci][:, 128 * mc:128 * mc + 128]),
                            rhs=r(wvT_sb[ci]),
                            start=(ci == 0), stop=(ci == 1),
                        )
                    nc.vector.tensor_add(v_sb[mc], vt, bv_sb)

            # ---- per-(h,n) additive bias: qhat . (bp+bk)_headslice ----
            if bias_on:
                with tc.tile_pool(name="ps_b", bufs=1, space="PSUM") as psb:
                    bt = psb.tile([NL, H], f32)
                    for h in range(H):
                        nc.tensor.matmul(
                            bt[:, h:h + 1], lhsT=r(qhat_sb[:, h, :]),
                            rhs=r(bpb_sb[:, h:h + 1]),
                            start=True, stop=True,
                        )
                    nc.scalar.copy(out=bias_nh, in_=bt)

            # ---- main loop: scores -> softmax -> transposed attn ----
            with (
                tc.tile_pool(name="ps_sc", bufs=2, space="PSUM") as ps_sc,
                tc.tile_pool(name="ps_tr", bufs=4, space="PSUM") as ps_tr,
                tc.tile_pool(name="ps_hid", bufs=1, space="PSUM") as ps_hid,
            ):
                hid = ps_hid.tile([128, C], f32)
                for g in range(GROUPS):
                    s_ps = ps_sc.tile([128, M], f32, tag="scores")
                    for q in range(NG // 4):
                        # quadrant q: 4 consecutive n rows -> PSUM rows 32q..32q+32
                        n0 = g * NG + q * 4
                        if q % 2 == 0:
                            # one DMA pair fetches 8 n rows (2 quadrants); the
                            # d-major DRAM layout gives 8KB contiguous rows
                            et = []
                            for ch in range(2):
                                t = embed_pool.tile([128, 8, M], bf16, tag=f"e{ch}", name=f"e{ch}")
                                src = bass.AP(
                                    tensor=embedT_d.tensor,
                                    offset=embedT_d.offset + ch * 128 * NL * M + n0 * M,
                                    ap=[[NL * M, 128], [M, 8], [1, M]],
                                )
                                nc.sync.dma_start(out=t, in_=src)
                                et.append(t)
                        o = s_ps[32 * q:32 * q + 32, :]
                        tpos = (0, 32 * q)
                        # scores_e: all 4 n's share the xkT rhs -> dense i=32
                        for ch in range(2):
                            nc.tensor.matmul(o, lhsT=r(qk_sb[ch][:, n0:n0 + 4, :]),
                                             rhs=r(xkT_sb[ch]),
                                             start=(ch == 0), stop=False,
                                             tile_position=tpos)
                        # scores_p: per-n, zero-padded i=32 accumulates onto
                        # that n's 8 rows only
                        for i in range(4):
                            n = n0 + i
                            for ch in range(2):
                                nc.tensor.matmul(
                                    o, lhsT=r(qp_pad[ch][:, n, :, :]),
                                    rhs=r(et[ch][:, (q % 2) * 4 + i, :]),
                                    start=False, stop=(i == 3 and ch == 1),
                                    tile_position=tpos)

                    # softmax over m (free dim), 16 n x 8 h on partitions
                    af_t = work.tile([128, M], f32, tag="af")
                    nc.gpsimd.dma_start(
                        out=af_t,
                        in_=bass.AP(tensor=af_d.tensor,
                                    offset=af_d.offset + g * NG * M,
                                    ap=[[M, NG], [0, H], [1, M]]),
                    )
                    tt = work.tile([128, M], f32, tag="T")
                    if bias_on:
                        bias_grp = small.tile([128, 1], f32, tag="bgrp")
                        nc.gpsimd.dma_start(out=bias_grp,
                                          in_=bias_nh[g * NG:(g + 1) * NG, :])
                        nc.vector.tensor_scalar(out=tt, in0=s_ps, scalar1=bias_grp,
                                                scalar2=None,
                                                op0=mybir.AluOpType.add)
                        nc.vector.tensor_mul(tt, tt, af_t)
                    else:
                        nc.vector.tensor_mul(tt, s_ps, af_t)
                    if pen_on:
                        nc.vector.tensor_add(tt, tt, pen_sb)
                    negmax = small.tile([128, 1], f32, tag="negmax")
                    nc.vector.reduce_max(negmax, tt, axis=mybir.AxisListType.X,
                                         negate=True)
                    ex = work.tile([128, M], bf16, tag="ex")
                    sums = small.tile([128, 1], f32, tag="sums")
                    nc.scalar.activation(out=ex, in_=tt, func=AF.Exp,
                                         bias=negmax, scale=1.0, accum_out=sums)
                    rcp = small.tile([128, 1], f32, tag="rcp")
                    nc.vector.reciprocal(rcp, sums)
                    nc.vector.tensor_scalar(out=ex, in0=ex, scalar1=rcp,
                                            scalar2=None,
                                            op0=mybir.AluOpType.mult)
                    for mc in range(4):
                        tp = ps_tr.tile([128, 128], bf16, tag="tp")
                        nc.tensor.transpose(tp, ex[:, 128 * mc:128 * mc + 128],
                                            ident)
                        nc.scalar.copy(
                            out=attnT_sb[mc][:, g * NG:(g + 1) * NG, :], in_=tp)
                    if g % 2 == 1:
                        # attn @ v for this pair of groups (32 query rows)
                        pr = g // 2
                        for h in range(H):
                            for mc in range(4):
                                nc.tensor.matmul(
                                    hid[32 * pr:32 * pr + 32, 32 * h:32 * h + 32],
                                    lhsT=r(attnT_sb[mc][:, 32 * pr:32 * pr + 32, h]),
                                    rhs=r(v_sb[mc][:, 32 * h:32 * h + 32]),
                                    start=(mc == 0), stop=(mc == 3),
                                    tile_position=(0, 32 * pr),
                                )

                nc.scalar.copy(out=hid_sb, in_=hid)
                nc.sync.dma_start(out=out_d, in_=hid_sb)

    nc.compile()
    return nc


def _get_program(bias_on: bool, pen_on: bool):
    key = (bias_on, pen_on)
    if key not in _PROGRAM_CACHE:
        _PROGRAM_CACHE[key] = _build_program(bias_on, pen_on)
    return _PROGRAM_CACHE[key]


def kernel(input_q, input_k, input_v, embed_qk, key_weights, key_masks,
           attention_factors, Wq, bq, Wk, bk, Wv, bv, Wp, bp):
    global LAST_RESULT
    from concourse.bass_utils import run_bass_kernel_spmd

    f = np.float32
    input_q = np.asarray(input_q, f)
    input_k = np.asarray(input_k, f)
    input_v = np.asarray(input_v, f)
    embed_qk = np.asarray(embed_qk, f)
    key_weights = np.asarray(key_weights, f)
    key_masks = np.asarray(key_masks)
    attention_factors = np.asarray(attention_factors, f)
    Wq, bq, Wk, bk, Wv, bv, Wp, bp = (np.asarray(x, f) for x in
                                      (Wq, bq, Wk, bk, Wv, bv, Wp, bp))

    bias_on = bool(np.any(bp + bk != 0.0))
    pen_on = bool(np.any(key_masks))
    nc = _get_program(bias_on, pen_on)

    # host-side staging (layout + bf16 narrowing for the matmul path)
    import ml_dtypes
    bf = ml_dtypes.bfloat16
    wqT = np.ascontiguousarray(Wq.T).astype(bf)
    wvT = np.ascontiguousarray(Wv.T).astype(bf)
    wkr = np.ascontiguousarray(Wk.reshape(H, HC, C).transpose(1, 0, 2)).astype(bf)
    wpr = np.ascontiguousarray(Wp.reshape(H, HC, C).transpose(1, 0, 2)).astype(bf)
    bqs = np.ascontiguousarray((bq * SCALE).reshape(H, HC).T)
    bvb = np.ascontiguousarray(np.broadcast_to(bv[None, :], (128, C)))
    if bias_on:
        bpb = np.ascontiguousarray((bp + bk).reshape(H, HC).T)
    if pen_on:
        pen = np.where(key_masks, np.float32(-1e30), np.float32(0.0)).astype(f)

    af_eff = attention_factors * key_weights[:, None, :]  # (B, N, M)

    in_maps = []
    for core in range(NCORES):
        b = core // (NCORES // B)
        n0 = (core % (NCORES // B)) * NL
        m = {
            "embedT": np.ascontiguousarray(
                embed_qk[b, n0:n0 + NL].transpose(2, 0, 1)).astype(bf),
            "xqT": np.ascontiguousarray(input_q[b, n0:n0 + NL].T).astype(bf),
            "xkT": np.ascontiguousarray(input_k[b].T).astype(bf),
            "xvT": np.ascontiguousarray(input_v[b].T).astype(bf),
            "af": np.ascontiguousarray(af_eff[b, n0:n0 + NL]),
            "WqT": wqT, "Wkr": wkr, "Wpr": wpr, "WvT": wvT,
            "bqs": bqs, "bvb": bvb,
        }
        if bias_on:
            m["bpb"] = bpb
        if pen_on:
            m["pen"] = np.ascontiguousarray(pen[b].reshape(1, M))
        in_maps.append(m)

    res = run_bass_kernel_spmd(nc, in_maps, core_ids=list(range(NCORES)),
                               trace=TRACE)
    LAST_RESULT = res

    out = np.empty((B, N, C), f)
    for core in range(NCORES):
        b = core // (NCORES // B)
        n0 = (core % (NCORES // B)) * NL
        out[b, n0:n0 + NL] = res.results[core]["out"]
    return out


# revision 24
# speedup vs baseline: 3.0422x; 1.1083x over previous
"""Multi-head attention with per-pair relative embeddings (nn_MultiHeadAttention),
SPMD across 8 TRN2 NeuronCores.

Reference computation (B=2, N=M=512, C=256, H=8, HC=32):
    q = (input_q @ Wq.T + bq)   -> (B,H,N,HC)
    k = (input_k @ Wk.T + bk)   -> (B,H,M,HC)
    v = (input_v @ Wv.T + bv)   -> (B,H,M,HC)
    p = (embed_qk @ Wp.T + bp)  -> (B,N,M,H,HC)
    scores = (einsum('bhnc,bnmhc->bhnm', q, p) + einsum('bhnc,bhmc->bhnm', q, k)) / sqrt(HC)
    scores = scores * attention_factors[:,None] * key_weights[:,None,None,:]
    scores = where(key_masks, -inf, scores)
    attn = softmax(scores, -1)
    out = einsum('bhnm,bhmc->bhnc', attn, v) -> (B, N, C)

Key algebraic rewrite (32x FLOP reduction): never materialize p.  With
qhat = q / sqrt(HC):
    scores_p[h,n,m] = sum_d qp[h,n,d] * embed_qk[n,m,d],
        qp[h,n,d] = sum_hc qhat[h*32+hc, n] * Wp[h*32+hc, d]
    scores_e[h,n,m] = sum_c qk[h,n,c] * input_k[m,c],
        qk[h,n,c] = sum_hc qhat[h*32+hc, n] * Wk[h*32+hc, c]
plus a per-(h,n) constant  qhat . (bp+bk)_headslice  which is handled as an
additive bias (zero for the given inputs, still implemented generically).

Sharding: core = b*4 + (block of 128 query rows n).  Each core streams its
67MB embed_qk slice (host-pretransposed to (n, C, M) so the contraction dim C
lands on SBUF partitions) and computes scores / softmax / attn@v locally.
"""

import math
import os
import sys

import numpy as np

sys.path.insert(0, "/opt/trn_rl_repo")

B, N, M, C, H = 2, 512, 512, 256, 8
HC = C // H
NCORES = 8
NL = N * B // NCORES  # 128 query rows per core
NG = 16               # n rows per softmax group (16*8 heads = 128 partitions)
GROUPS = NL // NG     # 8 groups
SCALE = 1.0 / math.sqrt(HC)

# set by test.py for profiling runs
TRACE = False
LAST_RESULT = None

_PROGRAM_CACHE = {}


def _build_program(bias_on: bool, pen_on: bool):
    """Build the (SPMD, identical-per-core) Bass program."""
    import concourse.bacc as bacc
    import concourse.bass as bass
    import concourse.tile as tile
    from concourse import mybir
    from concourse.masks import make_identity

    f32 = mybir.dt.float32
    bf16 = mybir.dt.bfloat16
    f8 = mybir.dt.float8e4
    AF = mybir.ActivationFunctionType

    nc = bacc.Bacc(
        "TRN2",
        target_bir_lowering=False,
        debug=False,
        enable_asserts=True,
        num_devices=NCORES,
    )

    def din(name, shape, dt=f32):
        return nc.dram_tensor(name, list(shape), dt, kind="ExternalInput").ap()

    embedT_d = din("embedT", (C, NL, M), f8)     # per-core embed slice, (d, n, m)
    xqT_d = din("xqT", (C, NL), bf16)              # input_q slice, transposed
    xkT_d = din("xkT", (C, M), bf16)               # input_k (this core's b), transposed
    xvT_d = din("xvT", (C, M), bf16)               # input_v (this core's b), transposed
    af_d = din("af", (NL, M))                # attention_factors * key_weights
    wqT_d = din("WqT", (C, C), bf16)               # Wq.T  (cin, cout)
    wkr_d = din("Wkr", (HC, H, C), bf16)           # Wk[h*HC+hc, c] as [hc, h, c]
    wpr_d = din("Wpr", (HC, H, C), bf16)           # Wp[h*HC+hc, d] as [hc, h, d]
    wvT_d = din("WvT", (C, C), bf16)               # Wv.T  (cin, cout)
    bqs_d = din("bqs", (HC, H))              # (bq * SCALE)[h*HC+hc] as [hc, h]
    bv_d = din("bvb", (128, C))              # bv broadcast to 128 partitions
    if bias_on:
        bpb_d = din("bpb", (HC, H))          # (bp+bk)[h*HC+hc] as [hc, h]
    if pen_on:
        pen_d = din("pen", (1, M))           # -1e30 where key_mask else 0
    out_d = nc.dram_tensor("out", [NL, C], f32, kind="ExternalOutput").ap()

    def r(ap):
        return ap

    with tile.TileContext(nc) as tc:
        with (
            tc.tile_pool(name="consts", bufs=1) as consts,
            tc.tile_pool(name="embed", bufs=6) as embed_pool,
            tc.tile_pool(name="work", bufs=2) as work,
            tc.tile_pool(name="small", bufs=2) as small,
        ):
            # ---- load constants ----
            ident = consts.tile([128, 128], bf16)
            make_identity(nc, ident)

            def load2(dram, cols, tag, dt=f32):
                ts = []
                for ch in range(2):
                    t = consts.tile([128, cols], dt, tag=f"{tag}{ch}", name=f"{tag}{ch}")
                    nc.sync.dma_start(out=t, in_=dram[128 * ch:128 * ch + 128, :])
                    ts.append(t)
                return ts

            wqT_sb = load2(wqT_d, C, "wqT", bf16)
            wvT_sb = load2(wvT_d, C, "wvT", bf16)
            xqT_sb = load2(xqT_d, NL, "xqT", bf16)
            xkT_sb = load2(xkT_d, M, "xkT", bf16)
            xvT_sb = load2(xvT_d, M, "xvT", bf16)
            wkr_sb = consts.tile([HC, H, C], bf16)
            nc.sync.dma_start(out=wkr_sb, in_=wkr_d)
            wpr_sb = consts.tile([HC, H, C], bf16)
            nc.sync.dma_start(out=wpr_sb, in_=wpr_d)
            bqs_sb = consts.tile([HC, H], f32)
            nc.sync.dma_start(out=bqs_sb, in_=bqs_d)
            bv_sb = consts.tile([128, C], f32)
            nc.sync.dma_start(out=bv_sb, in_=bv_d)
            if bias_on:
                bpb_sb = consts.tile([HC, H], f32)
                nc.sync.dma_start(out=bpb_sb, in_=bpb_d)
            if pen_on:
                pen_sb = consts.tile([128, M], f32)
                nc.sync.dma_start(
                    out=pen_sb,
                    in_=bass.AP(tensor=pen_d.tensor, offset=pen_d.offset,
                                ap=[[0, 128], [1, M]]),
                )

            qhat_sb = consts.tile([HC, H, NL], bf16)
            # qp_pad[*, s, h, n] = qp[h, n] if s == n%4 else 0 — a zero-padded
            # i=32 stationary operand so each n's scores_p matmul can target a
            # 32-aligned PSUM quadrant while touching only its own 8 rows.
            qp_pad = [consts.tile([128, NL, 4, H], f8, tag=f"qp{i}", name=f"qp{i}") for i in range(2)]
            qk_sb = [consts.tile([128, NL, H], bf16, tag=f"qk{i}", name=f"qk{i}") for i in range(2)]
            v_sb = [consts.tile([128, C], bf16, tag=f"v{i}", name=f"v{i}") for i in range(4)]
            attnT_sb = [consts.tile([128, NL, H], bf16, tag=f"at{i}", name=f"at{i}") for i in range(4)]
            hid_sb = consts.tile([128, C], f32)
            if bias_on:
                bias_nh = consts.tile([NL, H], f32)

            # ---- q projection: qhat[hc, h, n] = ((xq @ Wq.T + bq) * s).T ----
            with tc.tile_pool(name="ps_q", bufs=2, space="PSUM") as psq:
                for h in range(H):
                    qt = psq.tile([HC, NL], f32, tag="qt")
                    for ci in range(2):
                        nc.tensor.matmul(
                            qt,
                            lhsT=r(wqT_sb[ci][:, HC * h:HC * h + HC]),
                            rhs=r(xqT_sb[ci]),
                            start=(ci == 0), stop=(ci == 1),
                        )
                    nc.scalar.activation(
                        out=qhat_sb[:, h, :], in_=qt, func=AF.Identity,
                        bias=bqs_sb[:, h:h + 1], scale=SCALE * 64.0,
                    )

            # ---- qp / qk: per-head contraction of qhat with Wp / Wk rows ----
            nc.vector.memset(qp_pad[0], 0.0)
            nc.vector.memset(qp_pad[1], 0.0)
            with tc.tile_pool(name="ps_qpk", bufs=2, space="PSUM") as pqpk:
                for h in range(H):
                    rq = qhat_sb[:, h, :]  # (HC, NL)
                    for dc in range(2):
                        tk = pqpk.tile([128, NL], f32, tag="tk")
                        nc.tensor.matmul(
                            tk,
                            lhsT=r(wkr_sb[:, h, 128 * dc:128 * dc + 128]),
                            rhs=r(rq), start=True, stop=True,
                        )
                        nc.scalar.copy(out=qk_sb[dc][:, :, h], in_=tk)
                        tp = pqpk.tile([128, NL], f32, tag="tp")
                        nc.tensor.matmul(
                            tp,
                            lhsT=r(wpr_sb[:, h, 128 * dc:128 * dc + 128]),
                            rhs=r(rq), start=True, stop=True,
                        )
                        tp4 = tp.rearrange("p (n4 s) -> p n4 s", s=4)
                        dst4 = qp_pad[dc][:, :, :, h].rearrange(
                            "p (n4 s2) s -> p n4 s2 s", s2=4)
                        for s in range(4):
                            eng = nc.scalar if s < 2 else nc.vector
                            if s < 2:
                                nc.scalar.copy(out=dst4[:, :, s, s], in_=tp4[:, :, s])
                            else:
                                nc.vector.tensor_copy(out=dst4[:, :, s, s], in_=tp4[:, :, s])

            # ---- v projection: v[m, cout] (+ bv) ----
            with tc.tile_pool(name="ps_v", bufs=2, space="PSUM") as psv:
                for mc in range(4):
                    vt = psv.tile([128, C], f32, tag="vt")
                    for ci in range(2):
                        nc.tensor.matmul(
                            vt,
                            lhsT=r(xvT_sb[ci][:, 128 * mc:128 * mc + 128]),
                            rhs=r(wvT_sb[ci]),
                            start=(ci == 0), stop=(ci == 1),
                        )
                    nc.vector.tensor_add(v_sb[mc], vt, bv_sb)

            # ---- per-(h,n) additive bias: qhat . (bp+bk)_headslice ----
            if bias_on:
                with tc.tile_pool(name="ps_b", bufs=1, space="PSUM") as psb:
                    bt = psb.tile([NL, H], f32)
                    for h in range(H):
                        nc.tensor.matmul(
                            bt[:, h:h + 1], lhsT=r(qhat_sb[:, h, :]),
                            rhs=r(bpb_sb[:, h:h + 1]),
                            start=True, stop=True,
                        )
                    nc.scalar.copy(out=bias_nh, in_=bt)

            # ---- main loop: scores -> softmax -> transposed attn ----
            with (
                tc.tile_pool(name="ps_sc", bufs=2, space="PSUM") as ps_sc,
                tc.tile_pool(name="ps_tr", bufs=4, space="PSUM") as ps_tr,
                tc.tile_pool(name="ps_hid", bufs=1, space="PSUM") as ps_hid,
            ):
                hid = ps_hid.tile([128, C], f32)
                for g in range(GROUPS):
                    s_ps = ps_sc.tile([128, M], f32, tag="scores")
                    for q in range(NG // 4):
                        # quadrant q: 4 consecutive n rows -> PSUM rows 32q..32q+32
                        n0 = g * NG + q * 4
                        if q % 2 == 0:
                            # one DMA pair fetches 8 n rows (2 quadrants); the
                            # d-major DRAM layout gives 8KB contiguous rows
                            et = []
                            for ch in range(2):
                                t = embed_pool.tile([128, 8, M], f8, tag=f"e{ch}", name=f"e{ch}")
                                src = bass.AP(
                                    tensor=embedT_d.tensor,
                                    offset=embedT_d.offset + ch * 128 * NL * M + n0 * M,
                                    ap=[[NL * M, 128], [M, 8], [1, M]],
                                )
                                nc.sync.dma_start(out=t, in_=src)
                                et.append(t)
                        o = s_ps[32 * q:32 * q + 32, :]
                        tpos = (0, 32 * q)
                        # scores_e: all 4 n's share the xkT rhs -> dense i=32
                        for ch in range(2):
                            nc.tensor.matmul(o, lhsT=r(qk_sb[ch][:, n0:n0 + 4, :]),
                                             rhs=r(xkT_sb[ch]),
                                             start=(ch == 0), stop=False,
                                             tile_position=tpos)
                        # scores_p: per-n, zero-padded i=32 accumulates onto
                        # that n's 8 rows only
                        for i in range(4):
                            n = n0 + i
                            for ch in range(2):
                                nc.tensor.matmul(
                                    o, lhsT=r(qp_pad[ch][:, n, :, :]),
                                    rhs=r(et[ch][:, (q % 2) * 4 + i, :]),
                                    start=False, stop=(i == 3 and ch == 1),
                                    tile_position=tpos)

                    # softmax over m (free dim), 16 n x 8 h on partitions
                    af_t = work.tile([128, M], f32, tag="af")
                    nc.gpsimd.dma_start(
                        out=af_t,
                        in_=bass.AP(tensor=af_d.tensor,
                                    offset=af_d.offset + g * NG * M,
                                    ap=[[M, NG], [0, H], [1, M]]),
                    )
                    tt = work.tile([128, M], f32, tag="T")
                    if bias_on:
                        bias_grp = small.tile([128, 1], f32, tag="bgrp")
                        nc.gpsimd.dma_start(out=bias_grp,
                                          in_=bias_nh[g * NG:(g + 1) * NG, :])
                        nc.vector.tensor_scalar(out=tt, in0=s_ps, scalar1=bias_grp,
                                                scalar2=None,
                                                op0=mybir.AluOpType.add)
                        nc.vector.tensor_mul(tt, tt, af_t)
                    else:
                        nc.vector.tensor_mul(tt, s_ps, af_t)
                    if pen_on:
                        nc.vector.tensor_add(tt, tt, pen_sb)
                    negmax = small.tile([128, 1], f32, tag="negmax")
                    nc.vector.reduce_max(negmax, tt, axis=mybir.AxisListType.X,
                                         negate=True)
                    ex = work.tile([128, M], bf16, tag="ex")
                    sums = small.tile([128, 1], f32, tag="sums")
                    nc.scalar.activation(out=ex, in_=tt, func=AF.Exp,
                                         bias=negmax, scale=1.0, accum_out=sums)
                    rcp = small.tile([128, 1], f32, tag="rcp")
                    nc.vector.reciprocal(rcp, sums)
                    nc.vector.tensor_scalar(out=ex, in0=ex, scalar1=rcp,
                                            scalar2=None,
                                            op0=mybir.AluOpType.mult)
                    for mc in range(4):
                        tp = ps_tr.tile([128, 128], bf16, tag="tp")
                        nc.tensor.transpose(tp, ex[:, 128 * mc:128 * mc + 128],
                                            ident)
                        nc.scalar.copy(
                            out=attnT_sb[mc][:, g * NG:(g + 1) * NG, :], in_=tp)
                    if g % 2 == 1:
                        # attn @ v for this pair of groups (32 query rows)
                        pr = g // 2
                        for h in range(H):
                            for mc in range(4):
                                nc.tensor.matmul(
                                    hid[32 * pr:32 * pr + 32, 32 * h:32 * h + 32],
                                    lhsT=r(attnT_sb[mc][:, 32 * pr:32 * pr + 32, h]),
                                    rhs=r(v_sb[mc][:, 32 * h:32 * h + 32]),
                                    start=(mc == 0), stop=(mc == 3),
                                    tile_position=(0, 32 * pr),
                                )

                nc.scalar.copy(out=hid_sb, in_=hid)
                nc.sync.dma_start(out=out_d, in_=hid_sb)

    nc.compile()
    return nc


def _get_program(bias_on: bool, pen_on: bool):
    key = (bias_on, pen_on)
    if key not in _PROGRAM_CACHE:
        _PROGRAM_CACHE[key] = _build_program(bias_on, pen_on)
    return _PROGRAM_CACHE[key]


def kernel(input_q, input_k, input_v, embed_qk, key_weights, key_masks,
           attention_factors, Wq, bq, Wk, bk, Wv, bv, Wp, bp):
    global LAST_RESULT
    from concourse.bass_utils import run_bass_kernel_spmd

    f = np.float32
    input_q = np.asarray(input_q, f)
    input_k = np.asarray(input_k, f)
    input_v = np.asarray(input_v, f)
    embed_qk = np.asarray(embed_qk, f)
    key_weights = np.asarray(key_weights, f)
    key_masks = np.asarray(key_masks)
    attention_factors = np.asarray(attention_factors, f)
    Wq, bq, Wk, bk, Wv, bv, Wp, bp = (np.asarray(x, f) for x in
                                      (Wq, bq, Wk, bk, Wv, bv, Wp, bp))

    bias_on = bool(np.any(bp + bk != 0.0))
    pen_on = bool(np.any(key_masks))
    nc = _get_program(bias_on, pen_on)

    # host-side staging (layout + bf16 narrowing for the matmul path)
    import ml_dtypes
    bf = ml_dtypes.bfloat16
    f8 = ml_dtypes.float8_e4m3
    wqT = np.ascontiguousarray(Wq.T).astype(bf)
    wvT = np.ascontiguousarray(Wv.T).astype(bf)
    wkr = np.ascontiguousarray(Wk.reshape(H, HC, C).transpose(1, 0, 2)).astype(bf)
    wpr = np.ascontiguousarray(Wp.reshape(H, HC, C).transpose(1, 0, 2)).astype(bf)
    bqs = np.ascontiguousarray((bq * SCALE * 64.0).reshape(H, HC).T)
    bvb = np.ascontiguousarray(np.broadcast_to(bv[None, :], (128, C)))
    if bias_on:
        bpb = np.ascontiguousarray((bp + bk).reshape(H, HC).T)
    if pen_on:
        pen = np.where(key_masks, np.float32(-1e30), np.float32(0.0)).astype(f)

    af_eff = (attention_factors * key_weights[:, None, :]) / 64.0  # (B, N, M)

    in_maps = []
    for core in range(NCORES):
        b = core // (NCORES // B)
        n0 = (core % (NCORES // B)) * NL
        m = {
            "embedT": np.ascontiguousarray(
                embed_qk[b, n0:n0 + NL].transpose(2, 0, 1)).astype(f8),
            "xqT": np.ascontiguousarray(input_q[b, n0:n0 + NL].T).astype(bf),
            "xkT": np.ascontiguousarray(input_k[b].T).astype(bf),
            "xvT": np.ascontiguousarray(input_v[b].T).astype(bf),
            "af": np.ascontiguousarray(af_eff[b, n0:n0 + NL]),
            "WqT": wqT, "Wkr": wkr, "Wpr": wpr, "WvT": wvT,
            "bqs": bqs, "bvb": bvb,
        }
        if bias_on:
            m["bpb"] = bpb
        if pen_on:
            m["pen"] = np.ascontiguousarray(pen[b].reshape(1, M))
        in_maps.append(m)

    res = run_bass_kernel_spmd(nc, in_maps, core_ids=list(range(NCORES)),
                               trace=TRACE)
    LAST_RESULT = res

    out = np.empty((B, N, C), f)
    for core in range(NCORES):
        b = core // (NCORES // B)
        n0 = (core % (NCORES // B)) * NL
        out[b, n0:n0 + NL] = res.results[core]["out"]
    return out
